# revision 36
# baseline (speedup 1.0000x reference)
"""Trainium2 Bass kernel for nn_BatchSoftmaxNomax (batch contrastive softmax loss).

Math: scores[b,c,n,f] = <ner[b,n,:], face[c,f,:]>, logits = scores.mean((n,f)),
loss = -mean_b log_softmax(logits)[b,b].
Since the span-means are linear, logits[b,c] = <mean_n ner[b], mean_f face[c]>,
so the O(B^2*N^2*D) einsum collapses to two mean-reductions + a [B,D]x[D,B] matmul.

Sharding (8 cores, batch-sharded), two launches with a host-side gather between
them (a device AllGather costs ~55us of cross-rank launch-skew wait through this
runtime - measured - so two independent launches win). Each launch carries a
fixed ~10us of runtime cost (cold-start engine stall ~3.3us, framework barrier
rounds, ~1.5us DMA-completion-to-semaphore latency per gating transfer, drain/
barrier teardown), so both bodies are organized to overlap everything with the
input stream.

Launch A (per core, 32 batch rows): host packs both input slices as ONE fp8
tensor [128, 8192] (pure reshape: p = 4m + n//8, line = [j, d] span-major), so
all eight 128KB streaming DMA slices are 1KB-contiguous per partition and the two
HWDGE rings saturate ~330 GB/s combined. fp8 halves the stream vs bf16; the
mean-of-32 averages quantization noise (~1e-3 on the loss, gate is 2e-2).
Span-sum on PE in fp8 DoubleRow perf mode: each matmul consumes a j-pair
([128, 2, 512] moving against sel duplicated across the k-pair), so a tensor's
8-span reduction is 4 accumulating matmuls with no ldweights stalls. sel rides
the gpsimd SWDGE so the HWDGE rings start on payload immediately. PSUM->SBUF
cast copies split across DVE/ACT; per-tensor [32, 512] fp8 means DMA out as
soon as each chain finishes (fp8 keeps the host diag and the device rowsum on
the same quantized means, which drops the loss error to ~1.5e-4).

Host: gathers/transposes the means into per-core [nmt | fmt] fp8 [128, 1152]
(k-major) and computes the 256 diagonal dot products in f32 from the fp8 means.

Launch B (per core): ACT exp-table warm-up first, 2 contiguous DMAs (the sync
half carries nmt + the first two d-chunks and gates the first matmul), logits
[32, 256] via 2 accumulating fp8 DoubleRow matmuls, ACT exp with fused row-sum
accumulate, output padded to 128 f32/row (sub-512B HBM writes pay a ~2x RMW
completion penalty that costs ~3us on the exec tail).
Host: loss = -mean(diag - log(rowsum)).
"""

import ml_dtypes
import numpy as np
from contextlib import ExitStack

B = 256      # global batch
N1 = 32      # ner spans
N2 = 32      # face spans
D = 512      # embed dim
M = 8        # cores
BL = B // M  # local batch rows per core (32)
KD = D // 128  # d-chunks (4)
PJ = 8       # spans folded into each partition line
PCOLS = D * PJ          # 4096 fp8 bytes per partition per tensor
NTILE = PCOLS // 2      # 2048 — half-tensor DMA tile width

_CACHE = {}


def _emit_a(ctx, tc, data, sel4):
    from concourse import mybir

    nc = tc.nc
    f32 = mybir.dt.float32
    bf16 = mybir.dt.bfloat16
    fp8 = mybir.dt.float8e4

    consts = ctx.enter_context(tc.tile_pool(name="consts", bufs=1))
    chunks = ctx.enter_context(tc.tile_pool(name="chunks", bufs=1))
    work = ctx.enter_context(tc.tile_pool(name="work", bufs=1))
    mpsum = ctx.enter_context(tc.tile_pool(name="mpsum", bufs=2, space="PSUM"))

    # sel rides the gpsimd SWDGE so both HWDGE rings start on payload
    # immediately.
    sel_sb = consts.tile([128, 2 * BL], fp8)
    nc.gpsimd.dma_start(sel_sb[:], sel4)
    # 8 streaming slices, one j-pair each (128KB, 1KB contiguous per partition
    # line): finer slices pipeline receipts best (measured; 256KB slices
    # delayed both the first and last matmul gates by ~1us). Both rings
    # share the 16 SDMA engines at ~330 GB/s aggregate.
    JW = 2 * D
    tiles = []
    qs = [nc.sync, nc.scalar]
    for t in range(8):
        tl = chunks.tile([128, JW], fp8, tag=f"t{t}", name=f"t{t}")
        qs[t % 2].dma_start(tl[:], data[:, t * JW:(t + 1) * JW])
        tiles.append(tl)

    # Span-mean on PE in fp8 DoubleRow perf mode: each matmul consumes one
    # j-pair slice ([128, 2, 512] moving, sel duplicated across the k-pair),
    # so a tensor's 8-span sum is 4 accumulating matmuls at 2x throughput.
    sel_k = sel_sb[:].rearrange("p (k m) -> p k m", k=2)
    ps = [
        mpsum.tile([BL, D], f32, tag=f"ps{i}", name=f"ps{i}")
        for i in range(2)
    ]
    # fp8 means: halves the tail-critical PSUM->SBUF copies and the out DMA;
    # stage B consumes fp8 anyway and the loss error stays ~1e-3 (gate 2e-2).
    # One plain bass SBUF tensor (concrete AP) so ONE post-context raw
    # output DMA can ship both chains. One full-width cast per chain on
    # different engines: tile serializes readers of the same PSUM tile
    # (measured 86ns handoff), so splitting one chain across DVE+ACT
    # buys nothing; chain0->ACT runs early, chain1->DVE right after the
    # last matmul.
    m = nc.alloc_sbuf_tensor("m", [BL, 2 * D], fp8).ap()
    from concourse.mybir import MatmulPerfMode
    cast_engines = [nc.scalar, nc.vector]
    for i in range(2):
        for s in range(4):
            view = tiles[4 * i + s][:].rearrange("p (k d) -> p k d", k=2)
            nc.tensor.matmul(
                ps[i][:], sel_k, view,
                start=(s == 0), stop=(s == 3),
                perf_mode=MatmulPerfMode.DoubleRow,
            )
        eng = cast_engines[i]
        if eng is nc.scalar:
            eng.copy(m[:, i * D:(i + 1) * D], ps[i][:])
        else:
            eng.tensor_copy(m[:, i * D:(i + 1) * D], ps[i][:])
    return m


def _emit_b(ctx, tc, fmt, nmt):
    from concourse import mybir

    nc = tc.nc
    f32 = mybir.dt.float32
    bf16 = mybir.dt.bfloat16
    AF = mybir.ActivationFunctionType

    from concourse.mybir import MatmulPerfMode

    fp8 = mybir.dt.float8e4
    sbuf = ctx.enter_context(tc.tile_pool(name="work", bufs=1))
    lpsum = ctx.enter_context(tc.tile_pool(name="lpsum", bufs=2, space="PSUM"))

    # Warm the ACT exp table first thing on the scalar engine, before its DMA.
    warm_in = sbuf.tile([1, 1], f32)
    nc.vector.memset(warm_in[:], 0.0)
    warm_out = sbuf.tile([1, 1], f32)
    nc.scalar.activation(warm_out[:], warm_in[:], AF.Exp)

    NF = KD * BL + KD * B
    nf = sbuf.tile([128, NF], fp8)
    # chunk 0 = nmt + fmt k=0,1 (gates the first DoubleRow matmul).
    half = KD * BL + 2 * B
    nc.sync.dma_start(nf[:, :half], fmt[:, :half])
    nc.scalar.dma_start(nf[:, half:], fmt[:, half:])
    nt = nf[:, :KD * BL].rearrange("p (k m) -> p k m", k=KD)
    ff = nf[:, KD * BL:].rearrange("p (k g) -> p k g", k=KD)

    # Logits via 2 accumulating fp8 DoubleRow matmuls (k-pairs of d-chunks).
    lg = lpsum.tile([BL, B], f32)
    for kp in range(KD // 2):
        nc.tensor.matmul(
            lg[:], nt[:, 2 * kp:2 * kp + 2, :], ff[:, 2 * kp:2 * kp + 2, :],
            start=(kp == 0), stop=(kp == KD // 2 - 1),
            perf_mode=MatmulPerfMode.DoubleRow,
        )

    # rowsum[b] = sum_c exp(logits[b, c]) via ACT fused row-accumulate.
    # Padded to 128 f32/row: sub-512B HBM writes pay a RMW completion penalty.
    # Plain bass SBUF tensor (concrete AP) for the post-context raw out DMA.
    rs = nc.alloc_sbuf_tensor("rs", [BL, 128], f32).ap()
    nc.vector.memset(rs, 0.0)
    # e_sb is never read (only the fused accum matters; it sums pre-rounding
    # values) - fp8 quarters the ACT write bandwidth vs f32.
    e_sb = sbuf.tile([BL, B], fp8)
    nc.scalar.activation(e_sb[:], lg[:], AF.Exp, accum_out=rs[:, 0:1])
    return rs


def _emit_s(ctx, tc, a0, a1, face):
    """Single-launch body: core computes its 32 ner means, ALL 256 face
    means (replicated face input), logits row-block, exp + rowsums.

    PE runs at 0.65GHz for its first ~3us then ramps to 1.2GHz (measured),
    so the 36-matmul chain (~12.6us) roughly matches the 4.5MB fp8 input
    stream (~13us) and the whole thing is stream-paced."""
    from concourse import mybir
    from concourse.mybir import MatmulPerfMode

    nc = tc.nc
    f32 = mybir.dt.float32
    bf16 = mybir.dt.bfloat16
    fp8 = mybir.dt.float8e4
    AF = mybir.ActivationFunctionType
    DR = MatmulPerfMode.DoubleRow

    chunks = ctx.enter_context(tc.tile_pool(name="chunks", bufs=1))
    work = ctx.enter_context(tc.tile_pool(name="work", bufs=1))
    psum = ctx.enter_context(tc.tile_pool(name="psum", bufs=1, space="PSUM"))

    # Warm the ACT exp table early (scalar is otherwise idle here).
    warm_in = work.tile([1, 1], f32)
    nc.vector.memset(warm_in[:], 0.0)
    warm_out = work.tile([1, 1], f32)
    nc.scalar.activation(warm_out[:], warm_in[:], AF.Exp)

    # Input stream. a0 (sync): ner sel + ner j-pairs 0-1. a1 (scalar):
    # face identity-sel + 128x128 identity + ner j-pairs 2-3. Then the
    # 4MB face in 8 x 512KB quarters (4KB contiguous per partition line),
    # chain0's quarters first, alternating rings.
    t_a0 = chunks.tile([128, 64 + 2048], fp8, tag="a0", name="a0")
    nc.sync.dma_start(t_a0[:], a0)
    t_a1 = chunks.tile([128, 384 + 2048], fp8, tag="a1", name="a1")
    nc.scalar.dma_start(t_a1[:], a1)
    qs = [nc.sync, nc.scalar]
    fq = []
    for k in range(8):
        tl = chunks.tile([128, 4096], fp8, tag=f"f{k}", name=f"f{k}")
        qs[k % 2].dma_start(tl[:], face[:, k * 4096:(k + 1) * 4096])
        fq.append(tl)

    sel_k = t_a0[:, 0:64].rearrange("p (k m) -> p k m", k=2)
    fi_k = t_a1[:, 0:256].rearrange("p (k m) -> p k m", k=2)
    ident = t_a1[:, 256:384]

    # ner span-sum chain: 4 DR matmuls -> ps_n [32, 512].
    ps_n = psum.tile([BL, D], f32, tag="ps_n", name="ps_n")
    nviews = [
        t_a0[:, 64 + 1024 * j:64 + 1024 * (j + 1)] for j in range(2)
    ] + [
        t_a1[:, 384 + 1024 * j:384 + 1024 * (j + 1)] for j in range(2)
    ]
    for s in range(4):
        nc.tensor.matmul(
            ps_n[:], sel_k, nviews[s].rearrange("p (k d) -> p k d", k=2),
            start=(s == 0), stop=(s == 3), perf_mode=DR,
        )
    nm8 = work.tile([BL, D], fp8, tag="nm8", name="nm8")
    nc.vector.tensor_copy(nm8[:], ps_n[:])

    # face chains: identity/32 sel, M=128 -> ps_f[h] [128, 512].
    ps_f = [
        psum.tile([128, D], f32, tag=f"ps_f{h}", name=f"ps_f{h}")
        for h in range(2)
    ]
    # One tile per 128-column cast piece: a shared tile would make each
    # transpose wait for ALL four cast pieces (tile-level dep tracking).
    fm8 = [
        [
            work.tile([128, 128], fp8, tag=f"fm8_{h}_{dc}", name=f"fm8_{h}_{dc}")
            for dc in range(4)
        ]
        for h in range(2)
    ]
    fmt = work.tile([128, 4 * 256], fp8, tag="fmt", name="fmt")
    # transpose mode passes values through: out dtype must match input,
    # and fp8 transpose writes with element step 2 (hw quirk) - allocate
    # 2x columns and use stride-2 views.
    ps_nt = psum.tile([128, 256], fp8, tag="ps_nt", name="ps_nt")
    ps_t = [
        psum.tile([128, 256], fp8, tag=f"ps_t{j}", name=f"ps_t{j}")
        for j in range(2)
    ]

    def step2(ap):
        return ap.rearrange("p (c two) -> p c two", two=2)[:, :, 0]
    nmt8 = work.tile([128, 128], fp8, tag="nmt8", name="nmt8")

    def face_chain(h):
        for s in range(16):
            view = fq[4 * h + s // 4][:, 1024 * (s % 4):1024 * (s % 4 + 1)]
            nc.tensor.matmul(
                ps_f[h][:], fi_k, view.rearrange("p (k d) -> p k d", k=2),
                start=(s == 0), stop=(s == 15), perf_mode=DR,
            )

    def fm_pipeline(h):
        # Per d-chunk: cast ps_f[h] piece -> transpose -> copy into fmt.
        # Piecewise so the first transpose starts ~0.3us after the chain
        # stops; cast+copy of a piece stay on ONE engine (pieces 0,2 DVE;
        # 1,3 ACT) so each piece pays one cross-engine hop (PE), not two.
        for dc in range(4):
            src = ps_f[h][:, 128 * dc:128 * (dc + 1)]
            pt = step2(ps_t[dc % 2][:])
            dst = fmt[:, 256 * dc + 128 * h:256 * dc + 128 * h + 128]
            if dc % 2 == 0:
                nc.vector.tensor_copy(fm8[h][dc][:], src)
                nc.tensor.transpose(pt, fm8[h][dc][:], ident)
                nc.vector.tensor_copy(dst, pt)
            else:
                nc.scalar.copy(fm8[h][dc][:], src)
                nc.tensor.transpose(pt, fm8[h][dc][:], ident)
                nc.scalar.copy(dst, pt)

    face_chain(0)
    # nm transposes while fm8[0]'s cast lands: nm8 [32, 512] -> nmt8.
    nt2 = step2(ps_nt[:])
    for dc in range(4):
        nc.tensor.transpose(
            nt2[:, 32 * dc:32 * (dc + 1)],
            nm8[:, 128 * dc:128 * (dc + 1)],
            ident[0:32, 0:32],
        )
    nc.vector.tensor_copy(nmt8[:], nt2)
    fm_pipeline(0)
    face_chain(1)
    fm_pipeline(1)

    # logits row-block [32, 256]: per c-half, 2 accumulating DR matmuls
    # over d-chunk pairs; exp + rowsum per half (rs col 0 / col 1).
    # Separate PSUM/e tiles per half so half1's matmuls don't serialize
    # behind half0's exp (tile tracks deps per tile).
    nmt_k = nmt8[:].rearrange("p (dc m) -> p dc m", dc=4)
    fmt_k = fmt[:].rearrange("p (dc c) -> p dc c", dc=4)
    rs = nc.alloc_sbuf_tensor("rs_sb", [BL, 128], f32).ap()
    nc.vector.memset(rs, 0.0)
    # Diag comes from the bf16 exp values (host takes log of e[b,b]):
    # bf16 cannot underflow for any realistic logit, err ~4e-5.
    e8 = nc.alloc_sbuf_tensor("e8_sb", [BL, B], bf16).ap()
    for h2 in range(2):
        csl = slice(128 * h2, 128 * (h2 + 1))
        lg = psum.tile([BL, 128], f32, tag=f"lg{h2}", name=f"lg{h2}")
        for q in range(2):
            nc.tensor.matmul(
                lg[:],
                nmt_k[:, 2 * q:2 * q + 2, :],
                fmt_k[:, 2 * q:2 * q + 2, csl],
                start=(q == 0), stop=(q == 1), perf_mode=DR,
            )
        nc.scalar.activation(
            e8[:, csl], lg[:], AF.Exp, accum_out=rs[:, h2:h2 + 1],
        )
    return e8, rs


def _build_s():
    import concourse.tile as tile
    from concourse import bacc, mybir

    fp8 = mybir.dt.float8e4
    f32 = mybir.dt.float32
    bf16 = mybir.dt.bfloat16
    nc = bacc.Bacc("TRN2", target_bir_lowering=False, debug=False, num_devices=M)
    a0 = nc.dram_tensor("a0", [128, 64 + 2048], fp8, kind="ExternalInput").ap()
    a1 = nc.dram_tensor("a1", [128, 384 + 2048], fp8, kind="ExternalInput").ap()
    face = nc.dram_tensor("face", [128, 32768], fp8, kind="ExternalInput").ap()
    e8_d = nc.dram_tensor("e8", [BL, B], bf16, kind="ExternalOutput").ap()
    rs_d = nc.dram_tensor("rs", [BL, 128], f32, kind="ExternalOutput").ap()
    with tile.TileContext(nc) as tc:
        with ExitStack() as ctx:
            e8, rs = _emit_s(ctx, tc, a0, a1, face)
    # Fire-and-forget output DMAs (see _build_a); receipts land during the
    # walrus teardown.
    nc.sync.dma_start(e8_d, e8).then_inc(nc.alloc_semaphore("os0"), 16)
    nc.scalar.dma_start(rs_d, rs).then_inc(nc.alloc_semaphore("os1"), 16)
    nc.compile()
    return nc


def get_nc_s():
    if "s" not in _CACHE:
        _CACHE["s"] = _build_s()
    return _CACHE["s"]


def build_in_maps_s(face_j, ner_j):
    fp8 = ml_dtypes.float8_e4m3fn
    sel1 = np.zeros((128, BL), fp8)
    sel1[np.arange(128), np.arange(128) // 4] = np.float32(1.0 / N1)
    sel4 = np.concatenate([sel1, sel1], axis=1)  # [128, 64]
    fi = np.zeros((128, 256), np.float32)
    fi[np.arange(128), np.arange(128)] = 1.0 / N2
    fi[np.arange(128), 128 + np.arange(128)] = 1.0 / N2
    ident = np.eye(128, dtype=np.float32)
    fi8 = fi.astype(fp8)
    id8 = ident.astype(fp8)
    # face replicated to every core: [128, 32768] = two c-halves of
    # [128 rows, 32 spans * 512 d] in natural row-major order.
    ff = np.asarray(face_j, dtype=np.float32).reshape(B, N2 * D).astype(fp8)
    face = np.ascontiguousarray(
        np.concatenate([ff[:128], ff[128:]], axis=1)
    )
    maps = []
    for c in range(M):
        sl = slice(c * BL, (c + 1) * BL)
        nerp = _pack_a(ner_j[sl])  # [128, 4096]
        a0 = np.ascontiguousarray(np.concatenate([sel4, nerp[:, :2048]], axis=1))
        a1 = np.ascontiguousarray(
            np.concatenate([fi8, id8, nerp[:, 2048:]], axis=1)
        )
        maps.append({"a0": a0, "a1": a1, "face": face})
    return maps


def combine_s(results):
    loss_terms = []
    for c, r in enumerate(results):
        ed = r["e8"][np.arange(BL), 32 * c + np.arange(BL)].astype(np.float32)
        rs = r["rs"].astype(np.float32)
        diag = np.log(ed)  # l[b,b] = log(exp(l[b,b]) as fp8)
        rowsum = rs[:, 0] + rs[:, 1]
        loss_terms.append(diag - np.log(rowsum))
    return np.asarray(-np.mean(np.concatenate(loss_terms)), dtype=np.float32)


def _build_a():
    import concourse.tile as tile
    from concourse import bacc, mybir

    bf16 = mybir.dt.bfloat16
    fp8 = mybir.dt.float8e4
    nc = bacc.Bacc("TRN2", target_bir_lowering=False, debug=False, num_devices=M)
    data = nc.dram_tensor("data", [128, 2 * PCOLS], fp8, kind="ExternalInput").ap()
    sel4 = nc.dram_tensor("sel4", [128, 2 * BL], fp8, kind="ExternalInput").ap()
    means = nc.dram_tensor("means", [BL, 2 * D], fp8, kind="ExternalOutput").ap()
    with tile.TileContext(nc) as tc:
        with ExitStack() as ctx:
            m = _emit_a(ctx, tc, data, sel4)
    # Fire-and-forget output DMA AFTER the tile context: the tile-exit
    # barrier already orders it after the cast copies, and nothing waits
    # on its ~2us HBM write receipt - it completes during the ~7us walrus
    # semaphore-clear teardown instead of extending the body. The then_inc
    # satisfies walrus's "DGE must have sync info"; no waiter.
    nc.sync.dma_start(means, m).then_inc(nc.alloc_semaphore("out_sem"), 16)
    nc.compile()
    return nc


def _build_b():
    import concourse.tile as tile
    from concourse import bacc, mybir

    f32 = mybir.dt.float32
    bf16 = mybir.dt.bfloat16
    nc = bacc.Bacc("TRN2", target_bir_lowering=False, debug=False, num_devices=M)
    fp8 = mybir.dt.float8e4
    fmt = nc.dram_tensor("fmt", [128, KD * BL + KD * B], fp8, kind="ExternalInput").ap()
    nmt = None
    out = nc.dram_tensor("out", [BL, 128], f32, kind="ExternalOutput").ap()
    with tile.TileContext(nc) as tc:
        with ExitStack() as ctx:
            rs = _emit_b(ctx, tc, fmt, nmt)
    # Fire-and-forget output DMA (see _build_a).
    nc.sync.dma_start(out, rs).then_inc(nc.alloc_semaphore("out_sem"), 16)
    nc.compile()
    return nc


def get_nc_a():
    if "a" not in _CACHE:
        _CACHE["a"] = _build_a()
    return _CACHE["a"]


def get_nc_b():
    if "b" not in _CACHE:
        _CACHE["b"] = _build_b()
    return _CACHE["b"]


def _pack_a(x):
    # [32, 32, 512] -> [p = 4m + n//8, j = n%8, d] -> [128, 4096], j-major lines
    fp8 = ml_dtypes.float8_e4m3fn
    return np.asarray(x, dtype=np.float32).reshape(128, PCOLS).astype(fp8)


def build_in_maps_a(face_j, ner_j):
    bf16 = ml_dtypes.bfloat16
    sel1 = np.zeros((128, BL), ml_dtypes.float8_e4m3fn)
    sel1[np.arange(128), np.arange(128) // 4] = np.float32(1.0 / N1)
    sel4 = np.concatenate([sel1, sel1], axis=1)
    maps = []
    for c in range(M):
        sl = slice(c * BL, (c + 1) * BL)
        data = np.concatenate([_pack_a(ner_j[sl]), _pack_a(face_j[sl])], axis=1)
        maps.append({"data": np.ascontiguousarray(data), "sel4": sel4})
    return maps


def _t_km(x):
    # [rows, 512] -> [d' = 128, k*rows + r] (k-major columns), contiguous
    rows = x.shape[0]
    return np.ascontiguousarray(
        x.reshape(rows, KD, 128).transpose(2, 1, 0).reshape(128, KD * rows)
    )


def build_in_maps_b(results_a):
    fp8 = ml_dtypes.float8_e4m3fn
    nm = [r["means"][:, :D].astype(np.float32) for r in results_a]
    fm = [r["means"][:, D:].astype(np.float32) for r in results_a]
    fmt = _t_km(np.concatenate(fm, axis=0)).astype(fp8)
    return [
        {"fmt": np.ascontiguousarray(
            np.concatenate([_t_km(nm[c]).astype(fp8), fmt], axis=1))}
        for c in range(M)
    ]


def host_diag(results_a):
    # diag logit for core c's rows: <nm_c[i], fm_c[i]> in f32
    return np.concatenate(
        [
            (
                r["means"][:, :D].astype(np.float32)
                * r["means"][:, D:].astype(np.float32)
            ).sum(axis=1)
            for r in results_a
        ]
    )


def combine(results_a, results_b):
    diag = host_diag(results_a)
    rsum = np.concatenate([r["out"][:, 0] for r in results_b])
    return np.asarray(-np.mean(diag - np.log(rsum)), dtype=np.float32)


def _ensure_ntff_hook():
    """The agent image's antenv lacks axon_hooks; synthesize it and register the
    ctypes NTFF hook from trn_agent_boot so trace=True profiling works."""
    import sys
    import types

    try:
        from antenv.axon_hooks import get_axon_ntff_profile_hook  # noqa: F401

        return
    except ImportError:
        pass
    import antenv
    from trn_agent_boot.trn_boot import _ntff_profile_via_ctypes

    mod = types.ModuleType("antenv.axon_hooks")
    state = {"hook": None}
    mod.set_axon_ntff_profile_hook = lambda h: state.__setitem__("hook", h)
    mod.get_axon_ntff_profile_hook = lambda: state["hook"]
    sys.modules["antenv.axon_hooks"] = mod
    antenv.axon_hooks = mod
    mod.set_axon_ntff_profile_hook(_ntff_profile_via_ctypes("/opt/axon/libaxon_pjrt.so"))


def run_stage(nc, in_maps, trace=False, **kw):
    from concourse import bass_utils

    if trace:
        _ensure_ntff_hook()
    return bass_utils.run_bass_kernel_spmd(
        nc, in_maps, core_ids=list(range(M)), trace=trace, **kw
    )


SINGLE_LAUNCH = True


def kernel(face_j, ner_j):
    if SINGLE_LAUNCH:
        res = run_stage(get_nc_s(), build_in_maps_s(face_j, ner_j))
        return combine_s(res.results)
    res_a = run_stage(get_nc_a(), build_in_maps_a(face_j, ner_j))
    res_b = run_stage(get_nc_b(), build_in_maps_b(res_a.results))
    return combine(res_a.results, res_b.results)

